# revision 15
# baseline (speedup 1.0000x reference)
"""Distributed MoE layer (16384 tokens, hidden 1024, ffn 4096, 8 experts, top-2)
on 8 TRN2 NeuronCores, expert-parallel.

Host: router (replicated math, same semantics as the jax reference) + token
dispatch by target expert (the "all-to-all") + weighted combine.
Device (per core e): y = coeff * (gelu(x_e @ w1[e]) @ w2[e]) for the tokens
routed to expert e, in fp32r (full-rate fp32 matmul mode on TRN2).
"""

import sys

if "/opt/trn_rl_repo" not in sys.path:
    sys.path.insert(0, "/opt/trn_rl_repo")

import numpy as np

import concourse.mybir as mybir
import concourse.tile as tile
from concourse.tile import add_dep_helper
from concourse import bacc
from concourse.bass_utils import run_bass_kernel_spmd

N_TOKENS = 16384
HIDDEN = 1024
FFN = 4096
N_EXPERTS = 8
TOP_K = 2
P = 128
TG = 512  # token group (moving-operand width)

FP32 = mybir.dt.float32
FP32R = mybir.dt.float32r

_KERNEL_CACHE: dict[int, object] = {}


def _build(C: int):
    """Per-core FFN kernel for capacity C (multiple of 128)."""
    assert C % P == 0
    KH = HIDDEN // P  # 8 hidden chunks
    KF = FFN // P  # 32 ffn chunks
    groups = []
    base = 0
    while base < C:
        w = min(TG, C - base)
        groups.append((base, w))
        base += w
    nc = bacc.Bacc("TRN2", target_bir_lowering=False, debug=False)
    gelu = mybir.ActivationFunctionType.Gelu_apprx_tanh

    with tile.TileContext(nc) as tc:
        with tc.tile_pool(name="dram", bufs=1, space="DRAM") as dram:
            xt = dram.tile([HIDDEN, C], FP32R, kind="ExternalInput", uniquify=False, name="xt")
            w1 = dram.tile([FFN // P, P, HIDDEN // P, P], FP32R, kind="ExternalInput", uniquify=False, name="w1")
            w2 = dram.tile([FFN, HIDDEN], FP32R, kind="ExternalInput", uniquify=False, name="w2")
            cf = dram.tile([C, 1], FP32, kind="ExternalInput", uniquify=False, name="cf")
            y = dram.tile([C, HIDDEN], FP32, kind="ExternalOutput", uniquify=False, name="y")
            ht = dram.tile([FFN, C], FP32R, kind="Internal", uniquify=False, name="ht")

            xt3 = xt[:].rearrange("(ko p) n -> p ko n", p=P)  # [128, 8, C]
            w23 = w2[:].rearrange("(fo p) h -> p fo h", p=P)  # [128, 32, 1024]
            ht3 = ht[:].rearrange("(fo p) n -> p fo n", p=P)  # [128, 32, C]

            NEARLY = 6  # w2 chunks preloaded during phase 1 (dedicated space)
            w2ts = [None] * KF
            w2e_ctx = tc.tile_pool(name="w2e", bufs=1)
            w2e = w2e_ctx.__enter__()
            hip_ctx = tc.tile_pool(name="hip", bufs=6)
            hip = hip_ctx.__enter__()

            # ---- phase 1: hT = gelu(w1.T @ x.T), spilled to DRAM ----
            # w1 is loaded in per-k chunks so the first matmul starts after
            # ~4MB of DMA instead of 16MB; the first group's x slice is
            # queued first.
            with tc.tile_pool(name="w1p", bufs=1) as w1p, tc.tile_pool(
                name="xp", bufs=2
            ) as xp, tc.tile_pool(name="hp", bufs=3) as hp, tc.tile_pool(
                name="pp1", bufs=4, space="PSUM"
            ) as pp1:
                xg0 = xp.tile([P, KH, TG], FP32R, name="xg", tag="xg")
                nc.gpsimd.dma_start(xg0[:, :, : groups[0][1]], xt3[:, :, : groups[0][1]])
                w1ts = []
                for f in range(KF):
                    w1t = w1p.tile([P, KH, P], FP32R, name=f"w1t{f}", tag=f"w1t{f}")
                    nc.scalar.dma_start(w1t[:], w1[f])
                    w1ts.append(w1t)
                def gemm1_f(xg, base, w, f):
                    sl = slice(base, base + w)
                    ps_h = pp1.tile([P, TG], FP32, name="ps_h", tag="ps_h")
                    for k in range(KH):
                        nc.tensor.matmul(
                            ps_h[:, :w],
                            lhsT=w1ts[f][:, k, :],
                            rhs=xg[:, k, :w],
                            start=(k == 0),
                            stop=(k == KH - 1),
                        )
                    hg = hp.tile([P, TG], FP32R, name="hg", tag="hg")
                    nc.scalar.activation(hg[:, :w], ps_h[:, :w], gelu)
                    nc.sync.dma_start(ht3[:, f, sl], hg[:, :w])

                # The first two groups interleave over f so the w1 stream
                # only needs to deliver each chunk at half rate (group 0's
                # window would otherwise demand >HBM bandwidth).
                xgs = {0: xg0}
                nfused = min(2, len(groups))
                LAG = 10  # group 1 trails group 0 by LAG f-chunks
                for gi in range(1, nfused):
                    base, w = groups[gi]
                    xg = xp.tile([P, KH, TG], FP32R, name="xg", tag="xg")
                    nc.gpsimd.dma_start(xg[:, :, :w], xt3[:, :, base : base + w])
                    xgs[gi] = xg
                for f in range(KF + (LAG if nfused > 1 else 0)):
                    if f < KF:
                        gemm1_f(xgs[0], groups[0][0], groups[0][1], f)
                    if nfused > 1 and f >= LAG:
                        gemm1_f(xgs[1], groups[1][0], groups[1][1], f - LAG)
                    # early w2 chunks interleaved into the fused block tail
                    fe = f - (KF + (LAG if nfused > 1 else 0) - NEARLY)
                    if fe >= 0:
                        w2t = w2e.tile(
                            [P, HIDDEN], FP32R, name=f"w2t{fe}", tag=f"w2t{fe}"
                        )
                        nc.scalar.dma_start(w2t[:], w23[:, fe, :])
                        w2ts[fe] = w2t
                for gi in range(nfused, len(groups)):
                    base, w = groups[gi]
                    xg = xp.tile([P, KH, TG], FP32R, name="xg", tag="xg")
                    nc.gpsimd.dma_start(xg[:, :, :w], xt3[:, :, base : base + w])
                    for f in range(KF):
                        gemm1_f(xg, base, w, f)

            # ---- phase 2: y = coeff * (hT.T @ w2) ----
            # Remaining w2 chunks stream in f order into freed phase-1 space;
            # GEMM2 consumes them in the same order, so the load pipeline
            # stays ahead of the PE.
            with tc.tile_pool(name="w2p", bufs=1) as w2p, tc.tile_pool(
                name="cp", bufs=2
            ) as cp, tc.tile_pool(
                name="yp", bufs=2
            ) as yp, tc.tile_pool(name="pp2", bufs=1, space="PSUM") as pp2:
                def load_w2(f):
                    w2t = w2p.tile([P, HIDDEN], FP32R, name=f"w2t{f}", tag=f"w2t{f}")
                    nc.scalar.dma_start(w2t[:], w23[:, f, :])
                    w2ts[f] = w2t

                for gi, (base, w) in enumerate(groups):
                    sl = slice(base, base + w)
                    nt = w // P
                    cts = []
                    for tt in range(nt):
                        ct = cp.tile([P, 1], FP32, name="ct", tag=f"ct{tt}")
                        nc.gpsimd.dma_start(
                            ct[:], cf[base + tt * P : base + (tt + 1) * P, :]
                        )
                        cts.append(ct)
                    psy = [
                        [
                            pp2.tile([P, TG], FP32, name="psy", tag=f"psy{tt}_{nh}")
                            for nh in range(HIDDEN // TG)
                        ]
                        for tt in range(nt)
                    ]
                    for f in range(KF):
                        # stream the remaining w2 chunks just ahead of use so
                        # their DMAs interleave with hf loads in queue order
                        if gi == 0 and NEARLY + f < KF:
                            load_w2(NEARLY + f)
                        hf = hip.tile([P, TG], FP32R, name="hf", tag="hf")
                        nc.gpsimd.dma_start(hf[:, :w], ht3[:, f, sl])
                        for tt in range(nt):
                            for nh in range(HIDDEN // TG):
                                nc.tensor.matmul(
                                    psy[tt][nh][:],
                                    lhsT=hf[:, tt * P : (tt + 1) * P],
                                    rhs=w2ts[f][:, nh * TG : (nh + 1) * TG],
                                    start=(f == 0),
                                    stop=(f == KF - 1),
                                )
                    for tt in range(nt):
                        for nh in range(HIDDEN // TG):
                            yt = yp.tile([P, TG], FP32, name="yt", tag=f"yt{tt}_{nh}")
                            nc.vector.tensor_scalar_mul(yt[:], psy[tt][nh][:], cts[tt][:])
                            nc.sync.dma_start(
                                y[base + tt * P : base + (tt + 1) * P, nh * TG : (nh + 1) * TG],
                                yt[:],
                            )

            hip_ctx.__exit__(None, None, None)
            w2e_ctx.__exit__(None, None, None)

    nc.compile()
    return nc


def _get_kernel(C: int):
    if C not in _KERNEL_CACHE:
        _KERNEL_CACHE[C] = _build(C)
    return _KERNEL_CACHE[C]


def _route(x: np.ndarray, w_router: np.ndarray):
    """Replicates the reference router: softmax -> top-2 -> renormalize."""
    logits = x @ w_router  # [N, E] fp32
    order = np.argsort(-logits, axis=1, kind="stable")
    i1, i2 = order[:, 0], order[:, 1]
    l64 = logits.astype(np.float64)
    l64 -= l64.max(axis=1, keepdims=True)
    e = np.exp(l64)
    p = e / e.sum(axis=1, keepdims=True)
    rows = np.arange(x.shape[0])
    p1 = p[rows, i1]
    p2 = p[rows, i2]
    s = p1 + p2
    return i1, i2, (p1 / s).astype(np.float32), (p2 / s).astype(np.float32)


def kernel(x, w_router, w1, w2):
    x = np.ascontiguousarray(x, dtype=np.float32)
    w_router = np.ascontiguousarray(w_router, dtype=np.float32)
    w1 = np.ascontiguousarray(w1, dtype=np.float32)
    w2 = np.ascontiguousarray(w2, dtype=np.float32)
    n = x.shape[0]

    i1, i2, c1, c2 = _route(x, w_router)

    # dispatch: token lists per expert (the host-side all-to-all)
    slot_expert = np.concatenate([i1, i2])
    slot_coeff = np.concatenate([c1, c2])
    slot_token = np.concatenate([np.arange(n), np.arange(n)])
    counts = np.bincount(slot_expert, minlength=N_EXPERTS)
    C = int(max(P, -(-counts.max() // P) * P))

    order = np.argsort(slot_expert, kind="stable")
    tok_sorted = slot_token[order]
    coef_sorted = slot_coeff[order]
    starts = np.concatenate([[0], np.cumsum(counts)])

    tok_e = []
    in_maps = []
    for e in range(N_EXPERTS):
        te = tok_sorted[starts[e] : starts[e + 1]]
        ce = coef_sorted[starts[e] : starts[e + 1]]
        tok_e.append(te)
        xt = np.zeros((HIDDEN, C), dtype=np.float32)
        xt[:, : len(te)] = x[te].T
        cf = np.zeros((C, 1), dtype=np.float32)
        cf[: len(te), 0] = ce
        in_maps.append(
            {
                "xt": xt,
                "w1": np.ascontiguousarray(
                    w1[e].reshape(HIDDEN // P, P, FFN // P, P).transpose(2, 1, 0, 3)
                ),
                "w2": np.ascontiguousarray(w2[e]),
                "cf": cf,
            }
        )

    nc = _get_kernel(C)
    res = run_bass_kernel_spmd(nc, in_maps, core_ids=list(range(N_EXPERTS)))

    out = np.zeros((n, HIDDEN), dtype=np.float32)
    for e in range(N_EXPERTS):
        te = tok_e[e]
        out[te] += res.results[e]["y"][: len(te)]
    return out


# revision 17
# speedup vs baseline: 1.0004x; 1.0004x over previous
"""Distributed MoE layer (16384 tokens, hidden 1024, ffn 4096, 8 experts, top-2)
on 8 TRN2 NeuronCores, expert-parallel.

Host: router (replicated math, same semantics as the jax reference) + token
dispatch by target expert (the "all-to-all") + weighted combine.
Device (per core e): y = coeff * (gelu(x_e @ w1[e]) @ w2[e]) for the tokens
routed to expert e, in fp32r (full-rate fp32 matmul mode on TRN2).
"""

import sys

if "/opt/trn_rl_repo" not in sys.path:
    sys.path.insert(0, "/opt/trn_rl_repo")

import numpy as np

import concourse.mybir as mybir
import concourse.tile as tile
from concourse.tile import add_dep_helper
from concourse import bacc
from concourse.bass_utils import run_bass_kernel_spmd

N_TOKENS = 16384
HIDDEN = 1024
FFN = 4096
N_EXPERTS = 8
TOP_K = 2
P = 128
TG = 512  # token group (moving-operand width)

FP32 = mybir.dt.float32
FP32R = mybir.dt.float32r

_KERNEL_CACHE: dict[int, object] = {}


def _build(C: int):
    """Per-core FFN kernel for capacity C (multiple of 128)."""
    assert C % P == 0
    KH = HIDDEN // P  # 8 hidden chunks
    KF = FFN // P  # 32 ffn chunks
    groups = []
    base = 0
    while base < C:
        w = min(TG, C - base)
        groups.append((base, w))
        base += w
    nc = bacc.Bacc("TRN2", target_bir_lowering=False, debug=False)
    gelu = mybir.ActivationFunctionType.Gelu_apprx_tanh

    with tile.TileContext(nc) as tc:
        with tc.tile_pool(name="dram", bufs=1, space="DRAM") as dram:
            xt = dram.tile([HIDDEN, C], FP32R, kind="ExternalInput", uniquify=False, name="xt")
            w1 = dram.tile([FFN // P, P, HIDDEN // P, P], FP32R, kind="ExternalInput", uniquify=False, name="w1")
            w2 = dram.tile([FFN, HIDDEN], FP32R, kind="ExternalInput", uniquify=False, name="w2")
            cf = dram.tile([C, 1], FP32, kind="ExternalInput", uniquify=False, name="cf")
            y = dram.tile([C, HIDDEN], FP32, kind="ExternalOutput", uniquify=False, name="y")
            ht = dram.tile([FFN, C], FP32R, kind="Internal", uniquify=False, name="ht")

            xt3 = xt[:].rearrange("(ko p) n -> p ko n", p=P)  # [128, 8, C]
            w23 = w2[:].rearrange("(fo p) h -> p fo h", p=P)  # [128, 32, 1024]
            ht3 = ht[:].rearrange("(fo p) n -> p fo n", p=P)  # [128, 32, C]

            NEARLY = 6  # w2 chunks preloaded during phase 1 (dedicated space)
            w2ts = [None] * KF
            w2e_ctx = tc.tile_pool(name="w2e", bufs=1)
            w2e = w2e_ctx.__enter__()
            hip_ctx = tc.tile_pool(name="hip", bufs=6)
            hip = hip_ctx.__enter__()

            # ---- phase 1: hT = gelu(w1.T @ x.T), spilled to DRAM ----
            # w1 is loaded in per-k chunks so the first matmul starts after
            # ~4MB of DMA instead of 16MB; the first group's x slice is
            # queued first.
            with tc.tile_pool(name="w1p", bufs=1) as w1p, tc.tile_pool(
                name="xp", bufs=2
            ) as xp, tc.tile_pool(name="hp", bufs=5) as hp, tc.tile_pool(
                name="pp1", bufs=4, space="PSUM"
            ) as pp1:
                xg0 = xp.tile([P, KH, TG], FP32R, name="xg", tag="xg")
                nc.gpsimd.dma_start(xg0[:, :, : groups[0][1]], xt3[:, :, : groups[0][1]])
                w1ts = []
                for f in range(KF):
                    w1t = w1p.tile([P, KH, P], FP32R, name=f"w1t{f}", tag=f"w1t{f}")
                    nc.scalar.dma_start(w1t[:], w1[f])
                    w1ts.append(w1t)
                def gemm1_f(xg, base, w, f):
                    sl = slice(base, base + w)
                    ps_h = pp1.tile([P, TG], FP32, name="ps_h", tag="ps_h")
                    for k in range(KH):
                        nc.tensor.matmul(
                            ps_h[:, :w],
                            lhsT=w1ts[f][:, k, :],
                            rhs=xg[:, k, :w],
                            start=(k == 0),
                            stop=(k == KH - 1),
                        )
                    hg = hp.tile([P, TG], FP32R, name="hg", tag="hg")
                    nc.scalar.activation(hg[:, :w], ps_h[:, :w], gelu)
                    nc.sync.dma_start(ht3[:, f, sl], hg[:, :w])

                # The first two groups interleave over f so the w1 stream
                # only needs to deliver each chunk at half rate (group 0's
                # window would otherwise demand >HBM bandwidth).
                xgs = {0: xg0}
                nfused = min(2, len(groups))
                LAG = 6  # group 1 trails group 0 by LAG f-chunks
                for gi in range(1, nfused):
                    base, w = groups[gi]
                    xg = xp.tile([P, KH, TG], FP32R, name="xg", tag="xg")
                    nc.gpsimd.dma_start(xg[:, :, :w], xt3[:, :, base : base + w])
                    xgs[gi] = xg
                for f in range(KF + (LAG if nfused > 1 else 0)):
                    if f < KF:
                        gemm1_f(xgs[0], groups[0][0], groups[0][1], f)
                    if nfused > 1 and f >= LAG:
                        gemm1_f(xgs[1], groups[1][0], groups[1][1], f - LAG)
                    # early w2 chunks interleaved into the fused block tail
                    fe = f - (KF + (LAG if nfused > 1 else 0) - NEARLY)
                    if fe >= 0:
                        w2t = w2e.tile(
                            [P, HIDDEN], FP32R, name=f"w2t{fe}", tag=f"w2t{fe}"
                        )
                        nc.scalar.dma_start(w2t[:], w23[:, fe, :])
                        w2ts[fe] = w2t
                for gi in range(nfused, len(groups)):
                    base, w = groups[gi]
                    xg = xp.tile([P, KH, TG], FP32R, name="xg", tag="xg")
                    nc.gpsimd.dma_start(xg[:, :, :w], xt3[:, :, base : base + w])
                    for f in range(KF):
                        gemm1_f(xg, base, w, f)

            # ---- phase 2: y = coeff * (hT.T @ w2) ----
            # Remaining w2 chunks stream in f order into freed phase-1 space;
            # GEMM2 consumes them in the same order, so the load pipeline
            # stays ahead of the PE.
            with tc.tile_pool(name="w2p", bufs=1) as w2p, tc.tile_pool(
                name="cp", bufs=2
            ) as cp, tc.tile_pool(
                name="yp", bufs=2
            ) as yp, tc.tile_pool(name="pp2", bufs=1, space="PSUM") as pp2:
                def load_w2(f):
                    w2t = w2p.tile([P, HIDDEN], FP32R, name=f"w2t{f}", tag=f"w2t{f}")
                    nc.scalar.dma_start(w2t[:], w23[:, f, :])
                    w2ts[f] = w2t

                for gi, (base, w) in enumerate(groups):
                    sl = slice(base, base + w)
                    nt = w // P
                    cts = []
                    for tt in range(nt):
                        ct = cp.tile([P, 1], FP32, name="ct", tag=f"ct{tt}")
                        nc.gpsimd.dma_start(
                            ct[:], cf[base + tt * P : base + (tt + 1) * P, :]
                        )
                        cts.append(ct)
                    psy = [
                        [
                            pp2.tile([P, TG], FP32, name="psy", tag=f"psy{tt}_{nh}")
                            for nh in range(HIDDEN // TG)
                        ]
                        for tt in range(nt)
                    ]
                    for f in range(KF):
                        # stream the remaining w2 chunks just ahead of use so
                        # their DMAs interleave with hf loads in queue order
                        if gi == 0 and NEARLY + f < KF:
                            load_w2(NEARLY + f)
                        hf = hip.tile([P, TG], FP32R, name="hf", tag="hf")
                        nc.gpsimd.dma_start(hf[:, :w], ht3[:, f, sl])
                        for tt in range(nt):
                            for nh in range(HIDDEN // TG):
                                nc.tensor.matmul(
                                    psy[tt][nh][:],
                                    lhsT=hf[:, tt * P : (tt + 1) * P],
                                    rhs=w2ts[f][:, nh * TG : (nh + 1) * TG],
                                    start=(f == 0),
                                    stop=(f == KF - 1),
                                )
                    for tt in range(nt):
                        for nh in range(HIDDEN // TG):
                            yt = yp.tile([P, TG], FP32, name="yt", tag=f"yt{tt}_{nh}")
                            nc.vector.tensor_scalar_mul(yt[:], psy[tt][nh][:], cts[tt][:])
                            nc.sync.dma_start(
                                y[base + tt * P : base + (tt + 1) * P, nh * TG : (nh + 1) * TG],
                                yt[:],
                            )

            hip_ctx.__exit__(None, None, None)
            w2e_ctx.__exit__(None, None, None)

    nc.compile()
    return nc


def _get_kernel(C: int):
    if C not in _KERNEL_CACHE:
        _KERNEL_CACHE[C] = _build(C)
    return _KERNEL_CACHE[C]


def _route(x: np.ndarray, w_router: np.ndarray):
    """Replicates the reference router: softmax -> top-2 -> renormalize."""
    logits = x @ w_router  # [N, E] fp32
    order = np.argsort(-logits, axis=1, kind="stable")
    i1, i2 = order[:, 0], order[:, 1]
    l64 = logits.astype(np.float64)
    l64 -= l64.max(axis=1, keepdims=True)
    e = np.exp(l64)
    p = e / e.sum(axis=1, keepdims=True)
    rows = np.arange(x.shape[0])
    p1 = p[rows, i1]
    p2 = p[rows, i2]
    s = p1 + p2
    return i1, i2, (p1 / s).astype(np.float32), (p2 / s).astype(np.float32)


def kernel(x, w_router, w1, w2):
    x = np.ascontiguousarray(x, dtype=np.float32)
    w_router = np.ascontiguousarray(w_router, dtype=np.float32)
    w1 = np.ascontiguousarray(w1, dtype=np.float32)
    w2 = np.ascontiguousarray(w2, dtype=np.float32)
    n = x.shape[0]

    i1, i2, c1, c2 = _route(x, w_router)

    # dispatch: token lists per expert (the host-side all-to-all)
    slot_expert = np.concatenate([i1, i2])
    slot_coeff = np.concatenate([c1, c2])
    slot_token = np.concatenate([np.arange(n), np.arange(n)])
    counts = np.bincount(slot_expert, minlength=N_EXPERTS)
    C = int(max(P, -(-counts.max() // P) * P))

    order = np.argsort(slot_expert, kind="stable")
    tok_sorted = slot_token[order]
    coef_sorted = slot_coeff[order]
    starts = np.concatenate([[0], np.cumsum(counts)])

    tok_e = []
    in_maps = []
    for e in range(N_EXPERTS):
        te = tok_sorted[starts[e] : starts[e + 1]]
        ce = coef_sorted[starts[e] : starts[e + 1]]
        tok_e.append(te)
        xt = np.zeros((HIDDEN, C), dtype=np.float32)
        xt[:, : len(te)] = x[te].T
        cf = np.zeros((C, 1), dtype=np.float32)
        cf[: len(te), 0] = ce
        in_maps.append(
            {
                "xt": xt,
                "w1": np.ascontiguousarray(
                    w1[e].reshape(HIDDEN // P, P, FFN // P, P).transpose(2, 1, 0, 3)
                ),
                "w2": np.ascontiguousarray(w2[e]),
                "cf": cf,
            }
        )

    nc = _get_kernel(C)
    res = run_bass_kernel_spmd(nc, in_maps, core_ids=list(range(N_EXPERTS)))

    out = np.zeros((n, HIDDEN), dtype=np.float32)
    for e in range(N_EXPERTS):
        te = tok_e[e]
        out[te] += res.results[e]["y"][: len(te)]
    return out
